# revision 15
# baseline (speedup 1.0000x reference)
"""Trainium2 Bass kernel for nn_CauseEffectRepertoire.

Computes, for each of 2 directions (cause/effect) and batch b:
    min over masks m of KL(full_b || 0.5*(softmax(MLP(state_b*bits_m)) +
                                          softmax(MLP(state_b*(1-bits_m)))))
with D=16, H=64, B=8, M=2^15-1=32767 masks, via an 8-core SPMD kernel that
shards the mask axis (4096 masks per core, padded with one duplicate mask).

Math used on device (per mask m, batch b, direction):
    lin_a = bits_m @ (state_b[:,None]*w1.T) + b1        (mm1, bias folded in)
    lin_b = C - lin_a,  C = colsum(Wb) + 2*b1           (complement trick)
    la/lb = relu(lin) @ w2.T   (+ b2 folded into Exp bias)
    Ea = exp(la + b2), Za = sum_d Ea   (block-ones matmul)
    u_d = Ea_d*Zb + Eb_d*Za
    s = sum_d full_d*ln(u_d) - lnZa - lnZb              (block-diag matmuls)
    device returns max over masks of s  (per b, dir)
Host: KL_bits = (H - s_max)/ln2 + 1,  H = sum_d full ln full; min over cores.
"""

import os
import sys
from contextlib import ExitStack

import numpy as np

sys.path.insert(0, "/opt/trn_rl_repo")

D, H, B = 16, 64, 8
M = 2 ** (D - 1) - 1  # 32767
NCORES = 8
MPAD = 32768
MC = MPAD // NCORES  # 4096 masks per core
CHUNK = 512
NCHUNK = MC // CHUNK  # 8
LN2 = float(np.log(2.0))

_f32 = np.float32


def _mlp_softmax_np(x, w1, b1, w2, b2):
    h = np.maximum(x @ w1.T + b1, 0.0)
    lg = h @ w2.T + b2
    lg = lg - lg.max(axis=-1, keepdims=True)
    e = np.exp(lg)
    return e / e.sum(axis=-1, keepdims=True)


def _host_prep(inputs):
    """Build all device input arrays (float64 math, float32 outputs)."""
    state = np.asarray(inputs["state"], dtype=np.float64)  # (B, D)
    dirs = []
    for pre in ("cause", "effect"):
        dirs.append(
            tuple(
                np.asarray(inputs[f"{pre}_{k}"], dtype=np.float64)
                for k in ("w1", "b1", "w2", "b2")
            )
        )

    # mask bits, padded to MPAD with a duplicate of mask value 1
    mv = np.concatenate([np.arange(1, M + 1, dtype=np.int64), [1]])
    bits = ((mv[:, None] >> np.arange(D)[None, :]) & 1).astype(np.float64)  # (MPAD, D)

    # per-core bitsT (34, MC): [bits.T; ones; bits.T; ones]
    bitsT_cores = []
    for c in range(NCORES):
        bc = bits[c * MC : (c + 1) * MC].T  # (D, MC)
        ones = np.ones((1, MC))
        bitsT_cores.append(np.concatenate([bc, ones, bc, ones], axis=0).astype(_f32))

    # mm1 stationaries: (34, 8*128); col block idx8 = dir*4 + p packs b=(2p, 2p+1)
    mm1w = np.zeros((34, 8 * 128))
    cvec = np.zeros((128, 8))
    for d_ in range(2):
        w1, b1, w2, b2 = dirs[d_]
        for p in range(4):
            idx = d_ * 4 + p
            for half, b_ in enumerate((2 * p, 2 * p + 1)):
                Wb = state[b_][:, None] * w1.T  # (D, H)
                WbAug = np.concatenate([Wb, b1[None, :]], axis=0)  # (17, H)
                r0 = half * 17
                c0 = idx * 128 + half * 64
                mm1w[r0 : r0 + 17, c0 : c0 + 64] = WbAug
                cvec[half * 64 : half * 64 + 64, idx] = Wb.sum(axis=0) + 2.0 * b1

    # mm2 stationaries: (128, 64): per dir a (128, 32) block-diag of w2.T
    mm2w = np.zeros((128, 64))
    for d_ in range(2):
        w2T = dirs[d_][2].T  # (H, D)
        mm2w[0:64, d_ * 32 : d_ * 32 + 16] = w2T
        mm2w[64:128, d_ * 32 + 16 : d_ * 32 + 32] = w2T

    # Z-sum lhsT (128, 8): block ones
    zones = np.zeros((128, 8))
    for k in range(128):
        zones[k, k // 16] = 1.0

    # Z-broadcast lhsTs (40, 256): [bca | bcb]; Za rows 0-7, Zb rows 32-39
    bcsel = np.zeros((40, 256))
    for i in range(128):
        bcsel[i // 16, i] = 1.0
        bcsel[32 + i // 16, 128 + i] = 1.0

    # full softmax per dir + fmat/fsel + H consts
    fmat = np.zeros((128, 16))
    Hc = np.zeros((2, B))
    for d_ in range(2):
        w1, b1, w2, b2 = dirs[d_]
        full = _mlp_softmax_np(state, w1, b1, w2, b2)  # (B, D)
        Hc[d_] = (full * np.log(full)).sum(axis=1)
        for b_ in range(B):
            fmat[16 * b_ : 16 * b_ + 16, d_ * 8 + b_] = full[b_]

    fsel = np.zeros((40, 8))
    for b_ in range(B):
        fsel[b_, b_] = -1.0
        fsel[32 + b_, b_] = -1.0

    # Exp bias = b2 tiled per b: (128, 2)
    b2t = np.zeros((128, 2))
    for d_ in range(2):
        b2t[:, d_] = np.tile(dirs[d_][3], B)

    shared = {
        "mm1w": mm1w.astype(_f32),
        "cvec": cvec.astype(_f32),
        "mm2w": mm2w.astype(_f32),
        "zones": zones.astype(_f32),
        "bcsel": bcsel.astype(_f32),
        "fmat": fmat.astype(_f32),
        "fsel": fsel.astype(_f32),
        "b2t": b2t.astype(_f32),
    }
    in_maps = []
    for c in range(NCORES):
        m = dict(shared)
        m["bitsT"] = bitsT_cores[c]
        in_maps.append(m)
    return in_maps, Hc


_NC_CACHE = {}


def build_nc(repeats=1):
    """Build and compile the 8-core SPMD Bass program (cached).

    repeats>1 wraps the whole computation in a device-side loop — used only
    for benchmarking (amortizes host/tunnel dispatch overhead).
    """
    if repeats in _NC_CACHE:
        return _NC_CACHE[repeats]

    import concourse.bacc as bacc
    import concourse.bass as bass
    import concourse.tile as tile
    from concourse import mybir

    AF = mybir.ActivationFunctionType
    OP = mybir.AluOpType
    f32 = mybir.dt.float32

    nc = bacc.Bacc(
        "TRN2", target_bir_lowering=False, debug=False, num_devices=NCORES
    )

    ins = {}
    for name, shape in (
        ("bitsT", (34, MC)),
        ("mm1w", (34, 8 * 128)),
        ("cvec", (128, 8)),
        ("mm2w", (128, 64)),
        ("zones", (128, 8)),
        ("bcsel", (40, 256)),
        ("fmat", (128, 16)),
        ("fsel", (40, 8)),
        ("b2t", (128, 2)),
    ):
        ins[name] = nc.dram_tensor(name, shape, f32, kind="ExternalInput").ap()
    out_d = nc.dram_tensor("smax", (8, 2), f32, kind="ExternalOutput").ap()

    with tile.TileContext(nc) as tc, ExitStack() as ctx:
        cpool = ctx.enter_context(tc.tile_pool(name="consts", bufs=1))
        spool = ctx.enter_context(tc.tile_pool(name="work", bufs=3))
        pp_lin = ctx.enter_context(tc.tile_pool(name="plin", bufs=2, space="PSUM"))
        pp_log = ctx.enter_context(tc.tile_pool(name="plog", bufs=2, space="PSUM"))
        pp_zc = ctx.enter_context(tc.tile_pool(name="pzc", bufs=1, space="PSUM"))
        pp_ze = ctx.enter_context(tc.tile_pool(name="pze", bufs=2, space="PSUM"))
        pp_dot = ctx.enter_context(tc.tile_pool(name="pdot", bufs=1, space="PSUM"))

        # load constants
        ct = {}
        for name in ins:
            shp = list(ins[name].shape)
            t = cpool.tile(shp, f32, tag=name)
            nc.sync.dma_start(t[:], ins[name][:])
            ct[name] = t

        rep_ctx = tc.For_i(0, repeats, 1) if repeats > 1 else None
        if rep_ctx is not None:
            rep_ctx.__enter__()

        macc0 = cpool.tile([8, CHUNK], f32, tag="macc0")
        macc1 = cpool.tile([8, CHUNK], f32, tag="macc1")
        macc = [macc0, macc1]

        # persistent gappy Z tiles (rows 0-7 = Za, 32-39 = Zb; gap memset once
        # so the broadcast/correction matmuls never multiply 0 * garbage-NaN)
        ZCs0 = cpool.tile([40, CHUNK], f32, tag="ZCs0")
        ZCs1 = cpool.tile([40, CHUNK], f32, tag="ZCs1")
        lnZ0 = cpool.tile([40, CHUNK], f32, tag="lnZ0")
        lnZ1 = cpool.tile([40, CHUNK], f32, tag="lnZ1")
        ZCs_t = [ZCs0, ZCs1]
        lnZ_t = [lnZ0, lnZ1]
        for t in (ZCs0, ZCs1, lnZ0, lnZ1):
            nc.vector.memset(t[:], 0.0)

        for n in range(NCHUNK):
            for d_ in range(2):
                # ---- mm1 -> relu -> mm2, interleaved per b-pair ----
                La = pp_log.tile([128, CHUNK], f32, tag="logit")
                Lb = pp_log.tile([128, CHUNK], f32, tag="logit")
                for p in range(4):
                    idx = d_ * 4 + p
                    lt = pp_lin.tile([128, CHUNK], f32, tag="lin")
                    nc.tensor.matmul(
                        lt[:],
                        ct["mm1w"][:, idx * 128 : (idx + 1) * 128],
                        ct["bitsT"][:, n * CHUNK : (n + 1) * CHUNK],
                    )
                    # relu_a on DVE; relu_b = relu(C - lin) on ACT
                    ra = spool.tile([128, CHUNK], f32, tag="ra")
                    nc.vector.tensor_scalar(
                        ra[:], lt[:], 0.0, None, OP.max
                    )
                    rb = spool.tile([128, CHUNK], f32, tag="rb")
                    nc.scalar.activation(
                        rb[:],
                        lt[:],
                        AF.Relu,
                        bias=ct["cvec"][:, idx : idx + 1],
                        scale=-1.0,
                    )
                    w2blk = ct["mm2w"][:, d_ * 32 : (d_ + 1) * 32]
                    nc.tensor.matmul(
                        La[32 * p : 32 * p + 32, :], w2blk, ra[:],
                        start=True, stop=True,
                        tile_position=(0, 32 * p),
                    )
                    nc.tensor.matmul(
                        Lb[32 * p : 32 * p + 32, :], w2blk, rb[:],
                        start=True, stop=True,
                        tile_position=(0, 32 * p),
                    )
                # ---- exp (bias=b2) ----
                Ea = spool.tile([128, CHUNK], f32, tag="Ea")
                nc.scalar.activation(
                    Ea[:], La[:], AF.Exp, bias=ct["b2t"][:, d_ : d_ + 1]
                )
                Eb = spool.tile([128, CHUNK], f32, tag="Eb")
                nc.scalar.activation(
                    Eb[:], Lb[:], AF.Exp, bias=ct["b2t"][:, d_ : d_ + 1]
                )
                # ---- Z sums (PE) ----
                ZC = pp_zc.tile([64, CHUNK], f32, tag="zc")
                nc.tensor.matmul(ZC[0:8, :], ct["zones"][:], Ea[:],
                                 start=True, stop=True)
                nc.tensor.matmul(ZC[32:40, :], ct["zones"][:], Eb[:],
                                 start=True, stop=True, tile_position=(0, 32))
                # ---- evac Z (gappy 40 rows) + ln Z ----
                ZCs = ZCs_t[d_]
                nc.vector.tensor_copy(ZCs[0:8, :], ZC[0:8, :])
                nc.vector.tensor_copy(ZCs[32:40, :], ZC[32:40, :])
                lnZ = lnZ_t[d_]
                nc.scalar.activation(lnZ[0:8, :], ZC[0:8, :], AF.Ln)
                nc.scalar.activation(lnZ[32:40, :], ZC[32:40, :], AF.Ln)
                # ---- broadcast Z across (b,d) rows (PE) ----
                Zae = pp_ze.tile([128, CHUNK], f32, tag="ze")
                nc.tensor.matmul(Zae[:], ct["bcsel"][:, 0:128], ZCs[:])
                Zbe = pp_ze.tile([128, CHUNK], f32, tag="ze")
                nc.tensor.matmul(Zbe[:], ct["bcsel"][:, 128:256], ZCs[:])
                # ---- u = Ea*Zb + Eb*Za ----
                ua = spool.tile([128, CHUNK], f32, tag="ua")
                nc.vector.tensor_mul(ua[:], Ea[:], Zbe[:])
                ub = spool.tile([128, CHUNK], f32, tag="ub")
                nc.vector.tensor_mul(ub[:], Eb[:], Zae[:])
                u = spool.tile([128, CHUNK], f32, tag="u")
                nc.vector.tensor_add(u[:], ua[:], ub[:])
                lnU = spool.tile([128, CHUNK], f32, tag="lnU")
                nc.scalar.activation(lnU[:], u[:], AF.Ln)
                # ---- dot with full + lnZ correction (PE) ----
                dot = pp_dot.tile([8, CHUNK], f32, tag="dot")
                nc.tensor.matmul(dot[:], ct["fmat"][:, d_ * 8 : d_ * 8 + 8],
                                 lnU[:], start=True, stop=False)
                nc.tensor.matmul(dot[:], ct["fsel"][:], lnZ[:],
                                 start=False, stop=True)
                # ---- running max over masks ----
                if n == 0:
                    nc.vector.tensor_copy(macc[d_][:], dot[:])
                else:
                    nc.vector.tensor_max(macc[d_][:], macc[d_][:], dot[:])

        smax = spool.tile([8, 2], f32, tag="smax")
        for d_ in range(2):
            nc.vector.reduce_max(
                smax[:, d_ : d_ + 1], macc[d_][:], axis=mybir.AxisListType.X
            )
        nc.sync.dma_start(out_d[:], smax[:])

        if rep_ctx is not None:
            rep_ctx.__exit__(None, None, None)

    nc.compile()
    _NC_CACHE[repeats] = nc
    return nc


def kernel(**inputs):
    from concourse.bass_utils import run_bass_kernel_spmd

    in_maps, Hc = _host_prep(inputs)
    nc = build_nc()
    res = run_bass_kernel_spmd(nc, in_maps, list(range(NCORES)))
    s = np.max(np.stack([r["smax"] for r in res.results]), axis=0)  # (8, 2)
    # KL in bits: (H - s)/ln2 + 1 ; min over masks == max over s (done)
    kl = (Hc - s.T) / LN2 + 1.0  # (2, 8)
    return kl.astype(np.float32)


if __name__ == "__main__":
    import reference

    inp = reference.setup_inputs()
    inp = {k: np.asarray(v) for k, v in inp.items()}
    out = kernel(**inp)
    print(out)
